# revision 1
# baseline (speedup 1.0000x reference)
"""Trainium2 Bass kernel for MultiHeadSelfAttention.

Full inputs -> shard across 8 NeuronCores (batch x head-group) -> SPMD Bass
kernel -> gather partial projections on host.

Per-core shard (core c): batch b = c//4, head group g = c%4 (4 heads of 16).
Device program (per core, T=2048, D=1024, 4 heads, dh=64):
  phase A (per 512-col chunk): qT,kT = (Wqk.T @ x chunks) [feat,T] layout,
                               v     = x @ Wv (natural [T,feat]) + ones col
  phase B (per head, per tq-chunk): S^T[tk,tq] = kT.T-slice @ qT chunk,
    E = exp(S/8) * keepT (fp8 multiplicative mask, alternating DVE/GpSimd),
    O^T[65,tq] accumulated over tk tiles with a ones-row giving the softmax
    denominator d; normalize rows 0:64 by 1/d (K=1 broadcast matmul +
    fast reciprocal) and DMA-restack heads to [128, 2, T] for the projection.
    Two heads' chains are interleaved per group to hide cross-engine latency.
  phase C (per chunk): out[t,:] = O_cat^T.T @ Wp slices -> DMA out.
All matmuls run as float32r (full PE rate at free-dim>=256, ~fp32 precision).
"""

import os
import sys

import numpy as np

sys.path.insert(0, "/opt/trn_rl_repo")

import ml_dtypes  # noqa: E402

import concourse.mybir as mybir  # noqa: E402
import concourse.tile as tile  # noqa: E402
from concourse import bacc  # noqa: E402
from concourse.bass import ts  # noqa: E402

D = 1024  # d_model
HC = 4  # heads per core
DH = 64  # head dim
FQ = HC * DH  # 256: per-core q (or k or v) feature count
T_FULL = 2048
CHUNK = 512
NK = D // 128  # contraction tiles over d_model

F32 = mybir.dt.float32
F32R = mybir.dt.float32r
FP8 = mybir.dt.float8e4
EXP = mybir.ActivationFunctionType.Exp


def build_program(T=T_FULL, warm_mms=0):
    nch = T // CHUNK  # tq/t chunks
    ntt = T // 128  # t tiles
    tpc = CHUNK // 128  # t tiles per chunk

    nc = bacc.Bacc("TRN2", target_bir_lowering=False, debug=False)

    xt_d = nc.dram_tensor("xt", [D, T], F32R, kind="ExternalInput")
    wqk_d = nc.dram_tensor("wqk", [D, 2 * FQ], F32R, kind="ExternalInput")
    wv_d = nc.dram_tensor("wv", [D, FQ], F32R, kind="ExternalInput")
    wp_d = nc.dram_tensor("wp", [FQ, D], F32R, kind="ExternalInput")
    bqk_d = nc.dram_tensor("bqk", [128, 4], F32, kind="ExternalInput")
    bv_d = nc.dram_tensor("bv", [1, FQ], F32R, kind="ExternalInput")
    kp_d = nc.dram_tensor("keept", [T, T], FP8, kind="ExternalInput")
    out_d = nc.dram_tensor("out", [T, D], F32, kind="ExternalOutput")

    xt_r = xt_d.rearrange("(a p) t -> p a t", p=128)  # [128, NK, T]
    wqk_r = wqk_d.rearrange("(a p) f -> p a f", p=128)  # [128, NK, 512]
    wv_r = wv_d.rearrange("(a p) f -> p a f", p=128)  # [128, NK, 256]
    wp_r = wp_d.rearrange("(a p) n -> p a n", p=128)  # [128, 2, D]
    kp_r = kp_d.rearrange("(a p) q -> p a q", p=128)  # [128, ntt, T]

    with tile.TileContext(nc) as tc:
        with (
            tc.tile_pool(name="const", bufs=1) as constp,
            tc.tile_pool(name="stream", bufs=2) as streamp,
            tc.tile_pool(name="qkv", bufs=1) as qkvp,
            tc.tile_pool(name="work", bufs=2) as workp,
            tc.tile_pool(name="ps", bufs=2, space="PSUM") as psp,
        ):
            # ---- constants / weights ----
            ones_f = constp.tile([1, 128], F32, name="ones_f")
            nc.vector.memset(ones_f[:], 1.0)
            ones = constp.tile([1, 128], F32R, name="ones")
            nc.vector.tensor_copy(ones[:], ones_f[:])
            onec_f = constp.tile([128, 1], F32, name="onec_f")
            nc.vector.memset(onec_f[:], 1.0)
            # all-ones fp32r rows; row 64 pairs with the PSUM d-row base
            ones_col = constp.tile([128, DH], F32R, name="ones_col")
            nc.vector.tensor_copy(
                ones_col[:], onec_f[:, 0:1].broadcast_to([128, DH])
            )
            bqk_sb = constp.tile([128, 4], F32, name="bqk_sb")
            nc.sync.dma_start(bqk_sb[:], bqk_d[:, :])
            bv_sb = constp.tile([1, FQ], F32R, name="bv_sb")
            nc.sync.dma_start(bv_sb[:], bv_d[:, :])
            wqk_sb = constp.tile([128, NK, 2 * FQ], F32R, name="wqk_sb")
            nc.sync.dma_start(
                wqk_sb[:, 0 : NK // 2, :], wqk_r[:, 0 : NK // 2, :]
            )
            nc.gpsimd.dma_start(
                wqk_sb[:, NK // 2 :, :], wqk_r[:, NK // 2 :, :]
            )
            wv_sb = constp.tile([128, NK, FQ], F32R, name="wv_sb")
            nc.gpsimd.dma_start(wv_sb[:], wv_r[:, :, :])
            # broadcast v-bias row across partitions via K=1 matmul
            ps_bb = psp.tile([128, FQ], F32, tag="O", bufs=3, name="ps_bb")
            nc.tensor.matmul(
                ps_bb[:], (ones[0:1, 0:128]), (bv_sb[0:1, :]),
                start=True, stop=True,
            )
            bv_bc = constp.tile([128, FQ], F32, name="bv_bc")
            nc.vector.tensor_copy(bv_bc[:], ps_bb[:])

            ot_stk = constp.tile([128, FQ // 128, T], F32R, name="ot_stk")

            def attn_group(c, h, g, po, kp_t):
                hb = 64 * (h % 2)
                hf = h // 2
                ps_s = psp.tile([128, 2, CHUNK], F32, tag="S", name="ps_s")
                for j in range(2):
                    tkt = 2 * g + j
                    cc, tt = divmod(tkt, tpc)
                    nc.tensor.matmul(
                        ps_s[:, j, :],
                        (kt_tiles[cc][hb : hb + 64, hf, ts(tt, 128)]),
                        (qt_tiles[c][hb : hb + 64, hf, :]),
                        start=True, stop=True,
                    )
                e_t = workp.tile(
                    [128, 2, CHUNK], F32R, tag="e", bufs=6, name="e_t"
                )
                nc.scalar.activation(e_t[:], ps_s[:], EXP, scale=0.125)
                meng = nc.vector if h % 2 == 0 else nc.gpsimd
                meng.tensor_mul(e_t[:], e_t[:], kp_t[:, 2 * g : 2 * g + 2, :])
                for j in range(2):
                    tkt = 2 * g + j
                    cc, tt = divmod(tkt, tpc)
                    nc.tensor.matmul(
                        po[h][0 : DH + 1, :],
                        (v_tiles[cc][:, tt, h, :]),
                        (e_t[:, j, :]),
                        start=(tkt == 0), stop=(tkt == ntt - 1),
                    )

            def attn_tail(c, h, po):
                hb = 64 * (h % 2)
                hf = h // 2
                d_sb = workp.tile(
                    [128, CHUNK], F32R, tag="dsb", bufs=2, name="d_sb"
                )
                nc.vector.tensor_copy(
                    d_sb[DH : DH + 1, :], po[h][DH : DH + 1, :]
                )
                pb = psp.tile([64, CHUNK], F32, tag="F", bufs=1, name="pb")
                nc.tensor.matmul(
                    pb[:],
                    (ones_col[DH : DH + 1, :]),
                    (d_sb[DH : DH + 1, :]),
                    start=True, stop=True,
                )
                pb_sb = workp.tile(
                    [64, CHUNK], F32, tag="pbs", bufs=2, name="pb_sb"
                )
                nc.vector.tensor_copy(pb_sb[:], pb[:])
                rb = workp.tile([64, CHUNK], F32, tag="rb", bufs=2, name="rb")
                nc.vector.reciprocal_approx_fast(rb[:], pb_sb[:])
                ot_sb = workp.tile(
                    [64, CHUNK], F32R, tag="ot", bufs=2, name="ot_sb"
                )
                nc.vector.tensor_mul(ot_sb[:], po[h][0:DH, :], rb[:])
                nc.sync.dma_start(
                    ot_stk[hb : hb + 64, hf, ts(c, CHUNK)], ot_sb[:]
                )

            # chunk-0 pair-0 runs interleaved with phase A (ACT is idle there)
            kp0 = streamp.tile(
                [128, ntt, CHUNK], FP8, tag="kp", bufs=2, name="kp0"
            )
            nc.gpsimd.dma_start(kp0[:], kp_r[:, :, ts(0, CHUNK)])
            po0 = {}
            for h in (0, 1):
                po0[h] = psp.tile(
                    [128, CHUNK], F32, tag="O", bufs=3, name=f"po0_{h}"
                )

            # ---- phase A: qT, kT (transposed) and v (natural) ----
            kt_tiles, qt_tiles, v_tiles = [], [], []
            for c in range(nch):
                xt_t = streamp.tile(
                    [128, NK, CHUNK], F32R, tag="xt", bufs=2, name="xt_t"
                )
                nc.scalar.dma_start(
                    xt_t[:, 0 : NK // 2, :],
                    xt_r[:, 0 : NK // 2, ts(c, CHUNK)],
                )
                nc.sync.dma_start(
                    xt_t[:, NK // 2 :, :],
                    xt_r[:, NK // 2 :, ts(c, CHUNK)],
                )

                kt_t = qkvp.tile(
                    [128, 2, CHUNK], F32R, tag="kt", bufs=nch, name="kt_t"
                )
                qt_t = qkvp.tile(
                    [128, 2, CHUNK], F32R, tag="qt", bufs=nch, name="qt_t"
                )
                for f in range(2):  # k features: wqk cols 256..511
                    ps_k = psp.tile([128, CHUNK], F32, tag="S", name="ps_k")
                    for k in range(NK):
                        nc.tensor.matmul(
                            ps_k[:],
                            (wqk_sb[:, k, ts(2 + f, 128)]),
                            (xt_t[:, k, :]),
                            start=(k == 0), stop=(k == NK - 1),
                        )
                    nc.vector.tensor_scalar_add(
                        kt_t[:, f, :], ps_k[:], bqk_sb[:, 2 + f : 3 + f]
                    )
                for f in range(2):  # q features: wqk cols 0..255
                    ps_q = psp.tile([128, CHUNK], F32, tag="S", name="ps_q")
                    for k in range(NK):
                        nc.tensor.matmul(
                            ps_q[:],
                            (wqk_sb[:, k, ts(f, 128)]),
                            (xt_t[:, k, :]),
                            start=(k == 0), stop=(k == NK - 1),
                        )
                    nc.vector.tensor_scalar_add(
                        qt_t[:, f, :], ps_q[:], bqk_sb[:, f : f + 1]
                    )

                v_t = qkvp.tile(
                    [128, tpc, HC, DH + 1], F32R, tag="v", bufs=nch, name="v_t"
                )
                vcol = v_t[:, :, :, DH : DH + 1].rearrange(
                    "p a h e -> p (a h e)"
                )
                nc.vector.tensor_copy(
                    vcol, onec_f[:, 0:1].broadcast_to([128, tpc * HC])
                )
                for tt in range(tpc):
                    ps_v = psp.tile([128, FQ], F32, tag="F", bufs=1, name="ps_v")
                    for k in range(NK):
                        nc.tensor.matmul(
                            ps_v[:],
                            (xt_t[:, k, ts(tt, 128)]),
                            (wv_sb[:, k, :]),
                            start=(k == 0), stop=(k == NK - 1),
                        )
                    nc.vector.tensor_add(
                        v_t[:, tt, :, 0:DH],
                        ps_v[:, :].rearrange("p (h e) -> p h e", h=HC),
                        bv_bc[:, :].rearrange("p (h e) -> p h e", h=HC),
                    )
                kt_tiles.append(kt_t)
                qt_tiles.append(qt_t)
                v_tiles.append(v_t)
                for g in (2 * c, 2 * c + 1):
                    if g < ntt // 2:
                        for h in (0, 1):
                            attn_group(0, h, g, po0, kp0)

            for h in (0, 1):
                attn_tail(0, h, po0)

            # projection weights: first use is phase C, load after phase A
            wp_sb = constp.tile([128, FQ // 128, D], F32R, name="wp_sb")
            nc.sync.dma_start(wp_sb[:], wp_r[:, :, :])

            def emit_proj(cp):
                for tt in range(tpc):
                    tglob = cp * tpc + tt
                    o_t = workp.tile(
                        [128, D], F32, tag="out", bufs=2, name="o_t"
                    )
                    for n in range(D // CHUNK):
                        ps_f = psp.tile(
                            [128, CHUNK], F32, tag="F", bufs=1, name="ps_f"
                        )
                        for j in range(FQ // 128):
                            nc.tensor.matmul(
                                ps_f[:],
                                (ot_stk[:, j, ts(tglob, 128)]),
                                (wp_sb[:, j, ts(n, CHUNK)]),
                                start=(j == 0), stop=(j == FQ // 128 - 1),
                            )
                        nc.vector.tensor_copy(o_t[:, ts(n, CHUNK)], ps_f[:])
                    nc.sync.dma_start(out_d[ts(tglob, 128), :], o_t[:])

            # ---- phase B: attention per (chunk, head); phase C: projection
            # (each chunk's projection is emitted one chunk late, mid-pair,
            # so the PE never stalls on the normalize/restack chain) ----
            for c in range(nch):
                if c == 0:
                    kp_t = kp0
                else:
                    kp_t = streamp.tile(
                        [128, ntt, CHUNK], FP8, tag="kp", bufs=2, name="kp_t"
                    )
                    nc.gpsimd.dma_start(kp_t[:], kp_r[:, :, ts(c, CHUNK)])

                # two independent head chains interleaved per group: while
                # one head's tile sits in the ACT/DVE hop, the PE runs the
                # other head's matmuls (hides cross-engine latency)
                for hp in range(HC // 2):
                    if c == 0 and hp == 0:
                        continue  # interleaved into phase A
                    pair = (2 * hp, 2 * hp + 1)
                    po = {}
                    for h in pair:
                        po[h] = psp.tile(
                            [128, CHUNK], F32, tag="O", bufs=3, name=f"po{h}"
                        )
                    # software-pipeline: V-matmuls run one group behind the
                    # S->exp->mask chain, so the PE never waits on the exp or
                    # mask-mul it just fed (throughput- not latency-limited)
                    ngr = ntt // 2
                    vlag = 1
                    eprev = {}
                    for g in range(ngr + vlag):
                        for h in pair:
                            hb = 64 * (h % 2)
                            hf = h // 2
                            if g < ngr:
                                ps_s = psp.tile(
                                    [128, 2, CHUNK], F32, tag="S", name="ps_s"
                                )
                                for j in range(2):
                                    tkt = 2 * g + j
                                    cc, tt = divmod(tkt, tpc)
                                    nc.tensor.matmul(
                                        ps_s[:, j, :],
                                        (kt_tiles[cc][hb : hb + 64, hf, ts(tt, 128)]),
                                        (qt_tiles[c][hb : hb + 64, hf, :]),
                                        start=True, stop=True,
                                    )
                            if g >= vlag:
                                gp = g - vlag
                                e_p = eprev[(h, gp)]
                                for j in range(2):
                                    tkt = 2 * gp + j
                                    cc, tt = divmod(tkt, tpc)
                                    nc.tensor.matmul(
                                        po[h][0 : DH + 1, :],
                                        (v_tiles[cc][:, tt, h, :]),
                                        (e_p[:, j, :]),
                                        start=(tkt == 0), stop=(tkt == ntt - 1),
                                    )
                            if g < ngr:
                                e_t = workp.tile(
                                    [128, 2, CHUNK], F32R, tag="e", bufs=6,
                                    name="e_t",
                                )
                                nc.scalar.activation(
                                    e_t[:], ps_s[:], EXP, scale=0.125
                                )
                                # one head's mask-mul on DVE, one on GpSimd
                                meng = nc.vector if h % 2 == 0 else nc.gpsimd
                                meng.tensor_mul(
                                    e_t[:], e_t[:],
                                    kp_t[:, 2 * g : 2 * g + 2, :],
                                )
                                eprev[(h, g)] = e_t
                    for h in pair:
                        hb = 64 * (h % 2)
                        hf = h // 2
                        # normalize: broadcast d across dh partitions, 1/d, mul
                        d_sb = workp.tile(
                            [128, CHUNK], F32R, tag="dsb", bufs=2, name="d_sb"
                        )
                        nc.vector.tensor_copy(
                            d_sb[DH : DH + 1, :], po[h][DH : DH + 1, :]
                        )
                        pb = psp.tile(
                            [64, CHUNK], F32, tag="F", bufs=1, name="pb"
                        )
                        nc.tensor.matmul(
                            pb[:],
                            (ones_col[DH : DH + 1, :]),
                            (d_sb[DH : DH + 1, :]),
                            start=True, stop=True,
                        )
                        pb_sb = workp.tile(
                            [64, CHUNK], F32, tag="pbs", bufs=2, name="pb_sb"
                        )
                        nc.vector.tensor_copy(pb_sb[:], pb[:])
                        rb = workp.tile(
                            [64, CHUNK], F32, tag="rb", bufs=2, name="rb"
                        )
                        nc.vector.reciprocal_approx_fast(rb[:], pb_sb[:])
                        ot_sb = workp.tile(
                            [64, CHUNK], F32R, tag="ot", bufs=2, name="ot_sb"
                        )
                        nc.vector.tensor_mul(ot_sb[:], po[h][0:DH, :], rb[:])
                        # restack [64, h] -> [128, h//2] rows for the projection
                        nc.sync.dma_start(
                            ot_stk[hb : hb + 64, hf, ts(c, CHUNK)], ot_sb[:]
                        )

                emit_proj(c)


    nc.compile()
    return nc


def shard_inputs(x, mask, Wqkv, bqkv, Wproj):
    """Build the 8 per-core input maps from full inputs."""
    x = np.asarray(x, dtype=np.float32)
    Wqkv = np.asarray(Wqkv, dtype=np.float32)
    bqkv = np.asarray(bqkv, dtype=np.float32)
    keept = (np.asarray(mask)[0, 0].T == 0).astype(ml_dtypes.float8_e4m3)
    in_maps = []
    for c in range(8):
        b, g = divmod(c, 4)
        q0 = g * FQ
        wqk = np.concatenate(
            [Wqkv[:, q0 : q0 + FQ], Wqkv[:, D + q0 : D + q0 + FQ]], axis=1
        )
        bqk = np.concatenate(
            [bqkv[q0 : q0 + FQ], bqkv[D + q0 : D + q0 + FQ]]
        ).reshape(4, 128).T
        in_maps.append({
            "xt": np.ascontiguousarray(x[b].T),
            "wqk": np.ascontiguousarray(wqk),
            "wv": np.ascontiguousarray(Wqkv[:, 2 * D + q0 : 2 * D + q0 + FQ]),
            "wp": np.ascontiguousarray(np.asarray(Wproj, np.float32)[q0 : q0 + FQ, :]),
            "bqk": np.ascontiguousarray(bqk),
            "bv": bqkv[2 * D + q0 : 2 * D + q0 + FQ].reshape(1, FQ).copy(),
            "keept": keept,
        })
    return in_maps


_PROGRAM = None
WARM_MMS = int(os.environ.get("KERNEL_WARM_MMS", "0"))


def _get_program():
    global _PROGRAM
    if _PROGRAM is None:
        _PROGRAM = build_program(T_FULL, warm_mms=WARM_MMS)
    return _PROGRAM


def run_on_hw(in_maps, **kwargs):
    from concourse.bass_utils import run_bass_kernel_spmd

    nc = _get_program()
    return run_bass_kernel_spmd(nc, in_maps, list(range(8)), **kwargs)


def gather_output(results, bproj):
    parts = [results[c]["out"] for c in range(8)]
    out = np.stack([
        parts[0] + parts[1] + parts[2] + parts[3],
        parts[4] + parts[5] + parts[6] + parts[7],
    ])
    return (out + np.asarray(bproj, np.float32).reshape(1, 1, D)).astype(np.float32)


def kernel(x, mask, Wqkv, bqkv, Wproj, bproj):
    in_maps = shard_inputs(x, mask, Wqkv, bqkv, Wproj)
    res = run_on_hw(in_maps)
    return gather_output(res.results, bproj)



# revision 2
# speedup vs baseline: 1.2342x; 1.2342x over previous
"""Trainium2 Bass kernel for MultiHeadSelfAttention.

Full inputs -> shard across 8 NeuronCores (batch x head-group) -> SPMD Bass
kernel -> gather partial projections on host.

Per-core shard (core c): batch b = c//4, head group g = c%4 (4 heads of 16).
Device program (per core, T=2048, D=1024, 4 heads, dh=64):
  phase A (per 512-col chunk): qT,kT = (Wqk.T @ x chunks) [feat,T] layout,
                               v     = x @ Wv (natural [T,feat]) + ones col
  phase B (per head, per tq-chunk): S^T[tk,tq] = kT.T-slice @ qT chunk,
    E = exp(S/8) * keepT (bf16 multiplicative mask on DVE 2x-packed mode /
    GpSimd, load-balanced ~5:3), O^T[65,tq] accumulated over tk tiles with a
    ones-row giving the softmax denominator d; normalize rows 0:64 by 1/d
    (K=1 broadcast matmul + fast reciprocal) and DMA-restack heads to
    [128, 2, T] for the projection.
    Two heads' chains are interleaved per group to hide cross-engine latency;
    the K=64 score matmuls of the pair sit at base partitions 0/64 so they
    row-tile (64x128 mode) and run concurrently on the PE.
  phase C (per chunk): out[t,:] = O_cat^T.T @ Wp slices -> DMA out.
All matmuls run in bf16 (same 1 cyc/row as fp32r at free>=256 but half the
operand width/power -> avoids the K=4/8 HAM power throttle that capped the
fp32r version at ~1.2GHz effective, and enables FWL weight-load hiding).
PSUM accumulation stays fp32; softmax denominator path stays fp32.
"""

import os
import sys

import numpy as np

sys.path.insert(0, "/opt/trn_rl_repo")

import ml_dtypes  # noqa: E402

import concourse.mybir as mybir  # noqa: E402
import concourse.tile as tile  # noqa: E402
from concourse import bacc  # noqa: E402
from concourse.bass import ts  # noqa: E402

D = 1024  # d_model
HC = 4  # heads per core
DH = 64  # head dim
FQ = HC * DH  # 256: per-core q (or k or v) feature count
T_FULL = 2048
CHUNK = 512
NK = D // 128  # contraction tiles over d_model

F32 = mybir.dt.float32
F32R = mybir.dt.float32r
BF = mybir.dt.bfloat16
EXP = mybir.ActivationFunctionType.Exp


def _mask_eng(nc, h, g):
    # ~5:3 DVE:GpSimd split of the mask multiplies (GpSimd is ~2.5x slower
    # per element than DVE 2x-packed bf16; DVE also owns the PSUM-side ops)
    return nc.gpsimd if ((g + 3 * h) % 8) >= 5 else nc.vector


def build_program(T=T_FULL, warm_mms=0):
    nch = T // CHUNK  # tq/t chunks
    ntt = T // 128  # t tiles
    tpc = CHUNK // 128  # t tiles per chunk

    nc = bacc.Bacc("TRN2", target_bir_lowering=False, debug=False)

    xt_d = nc.dram_tensor("xt", [D, T], BF, kind="ExternalInput")
    wqk_d = nc.dram_tensor("wqk", [D, 2 * FQ], BF, kind="ExternalInput")
    wv_d = nc.dram_tensor("wv", [D, FQ], BF, kind="ExternalInput")
    wp_d = nc.dram_tensor("wp", [FQ, D], BF, kind="ExternalInput")
    bqk_d = nc.dram_tensor("bqk", [128, 4], F32, kind="ExternalInput")
    bv_d = nc.dram_tensor("bv", [1, FQ], F32R, kind="ExternalInput")
    kp_d = nc.dram_tensor("keept", [T, T], BF, kind="ExternalInput")
    out_d = nc.dram_tensor("out", [T, D], F32, kind="ExternalOutput")

    xt_r = xt_d.rearrange("(a p) t -> p a t", p=128)  # [128, NK, T]
    wqk_r = wqk_d.rearrange("(a p) f -> p a f", p=128)  # [128, NK, 512]
    wv_r = wv_d.rearrange("(a p) f -> p a f", p=128)  # [128, NK, 256]
    wp_r = wp_d.rearrange("(a p) n -> p a n", p=128)  # [128, 2, D]
    kp_r = kp_d.rearrange("(a p) q -> p a q", p=128)  # [128, ntt, T]

    with tile.TileContext(nc) as tc:
        with (
            tc.tile_pool(name="const", bufs=1) as constp,
            tc.tile_pool(name="stream", bufs=2) as streamp,
            tc.tile_pool(name="qkv", bufs=1) as qkvp,
            tc.tile_pool(name="work", bufs=2) as workp,
            tc.tile_pool(name="ps", bufs=2, space="PSUM") as psp,
        ):
            # ---- constants / weights ----
            ones_f = constp.tile([1, 128], F32, name="ones_f")
            nc.vector.memset(ones_f[:], 1.0)
            ones = constp.tile([1, 128], F32R, name="ones")
            nc.vector.tensor_copy(ones[:], ones_f[:])
            onec_f = constp.tile([128, 1], F32, name="onec_f")
            nc.vector.memset(onec_f[:], 1.0)
            # all-ones fp32r rows; row 64 pairs with the PSUM d-row base
            ones_col = constp.tile([128, DH], F32R, name="ones_col")
            nc.vector.tensor_copy(
                ones_col[:], onec_f[:, 0:1].broadcast_to([128, DH])
            )
            bqk_sb = constp.tile([128, 4], F32, name="bqk_sb")
            nc.sync.dma_start(bqk_sb[:], bqk_d[:, :])
            bv_sb = constp.tile([1, FQ], F32R, name="bv_sb")
            nc.sync.dma_start(bv_sb[:], bv_d[:, :])
            wqk_sb = constp.tile([128, NK, 2 * FQ], BF, name="wqk_sb")
            nc.sync.dma_start(
                wqk_sb[:, 0 : NK // 2, :], wqk_r[:, 0 : NK // 2, :]
            )
            nc.gpsimd.dma_start(
                wqk_sb[:, NK // 2 :, :], wqk_r[:, NK // 2 :, :]
            )
            wv_sb = constp.tile([128, NK, FQ], BF, name="wv_sb")
            nc.gpsimd.dma_start(wv_sb[:], wv_r[:, :, :])
            # broadcast v-bias row across partitions via K=1 matmul
            ps_bb = psp.tile([128, FQ], F32, tag="O", bufs=3, name="ps_bb")
            nc.tensor.matmul(
                ps_bb[:], (ones[0:1, 0:128]), (bv_sb[0:1, :]),
                start=True, stop=True,
            )
            bv_bc = constp.tile([128, FQ], F32, name="bv_bc")
            nc.vector.tensor_copy(bv_bc[:], ps_bb[:])

            ot_stk = constp.tile([128, FQ // 128, T], BF, name="ot_stk")

            def attn_group(c, h, g, po, kp_t):
                hb = 64 * (h % 2)
                hf = h // 2
                ps_s = psp.tile([128, 2, CHUNK], F32, tag="S", name="ps_s")
                for j in range(2):
                    tkt = 2 * g + j
                    cc, tt = divmod(tkt, tpc)
                    nc.tensor.matmul(
                        ps_s[:, j, :],
                        (kt_tiles[cc][hb : hb + 64, hf, ts(tt, 128)]),
                        (qt_tiles[c][hb : hb + 64, hf, :]),
                        start=True, stop=True,
                    )
                e_t = workp.tile(
                    [128, 2, CHUNK], BF, tag="e", bufs=6, name="e_t"
                )
                nc.scalar.activation(e_t[:], ps_s[:], EXP, scale=0.125)
                meng = _mask_eng(nc, h, g)
                meng.tensor_mul(e_t[:], e_t[:], kp_t[:, 2 * g : 2 * g + 2, :])
                for j in range(2):
                    tkt = 2 * g + j
                    cc, tt = divmod(tkt, tpc)
                    nc.tensor.matmul(
                        po[h][0 : DH + 1, :],
                        (v_tiles[cc][:, tt, h, :]),
                        (e_t[:, j, :]),
                        start=(tkt == 0), stop=(tkt == ntt - 1),
                    )

            def attn_tail(c, h, po):
                hb = 64 * (h % 2)
                hf = h // 2
                d_sb = workp.tile(
                    [128, CHUNK], F32R, tag="dsb", bufs=2, name="d_sb"
                )
                nc.vector.tensor_copy(
                    d_sb[DH : DH + 1, :], po[h][DH : DH + 1, :]
                )
                pb = psp.tile([64, CHUNK], F32, tag="F", bufs=1, name="pb")
                nc.tensor.matmul(
                    pb[:],
                    (ones_col[DH : DH + 1, :]),
                    (d_sb[DH : DH + 1, :]),
                    start=True, stop=True,
                )
                pb_sb = workp.tile(
                    [64, CHUNK], F32, tag="pbs", bufs=2, name="pb_sb"
                )
                nc.vector.tensor_copy(pb_sb[:], pb[:])
                rb = workp.tile([64, CHUNK], F32, tag="rb", bufs=2, name="rb")
                nc.vector.reciprocal_approx_fast(rb[:], pb_sb[:])
                ot_sb = workp.tile(
                    [64, CHUNK], BF, tag="ot", bufs=2, name="ot_sb"
                )
                nc.vector.tensor_mul(ot_sb[:], po[h][0:DH, :], rb[:])
                nc.sync.dma_start(
                    ot_stk[hb : hb + 64, hf, ts(c, CHUNK)], ot_sb[:]
                )

            # chunk-0 pair-0 runs interleaved with phase A (ACT is idle there)
            kp0 = streamp.tile(
                [128, ntt, CHUNK], BF, tag="kp", bufs=2, name="kp0"
            )
            nc.gpsimd.dma_start(kp0[:], kp_r[:, :, ts(0, CHUNK)])
            po0 = {}
            for h in (0, 1):
                po0[h] = psp.tile(
                    [128, CHUNK], F32, tag="O", bufs=3, name=f"po0_{h}"
                )

            # ---- phase A: qT, kT (transposed) and v (natural) ----
            kt_tiles, qt_tiles, v_tiles = [], [], []
            for c in range(nch):
                xt_t = streamp.tile(
                    [128, NK, CHUNK], BF, tag="xt", bufs=2, name="xt_t"
                )
                nc.scalar.dma_start(
                    xt_t[:, 0 : NK // 2, :],
                    xt_r[:, 0 : NK // 2, ts(c, CHUNK)],
                )
                nc.sync.dma_start(
                    xt_t[:, NK // 2 :, :],
                    xt_r[:, NK // 2 :, ts(c, CHUNK)],
                )

                kt_t = qkvp.tile(
                    [128, 2, CHUNK], BF, tag="kt", bufs=nch, name="kt_t"
                )
                qt_t = qkvp.tile(
                    [128, 2, CHUNK], BF, tag="qt", bufs=nch, name="qt_t"
                )
                for f in range(2):  # k features: wqk cols 256..511
                    ps_k = psp.tile([128, CHUNK], F32, tag="S", name="ps_k")
                    for k in range(NK):
                        nc.tensor.matmul(
                            ps_k[:],
                            (wqk_sb[:, k, ts(2 + f, 128)]),
                            (xt_t[:, k, :]),
                            start=(k == 0), stop=(k == NK - 1),
                        )
                    nc.vector.tensor_scalar_add(
                        kt_t[:, f, :], ps_k[:], bqk_sb[:, 2 + f : 3 + f]
                    )
                for f in range(2):  # q features: wqk cols 0..255
                    ps_q = psp.tile([128, CHUNK], F32, tag="S", name="ps_q")
                    for k in range(NK):
                        nc.tensor.matmul(
                            ps_q[:],
                            (wqk_sb[:, k, ts(f, 128)]),
                            (xt_t[:, k, :]),
                            start=(k == 0), stop=(k == NK - 1),
                        )
                    nc.vector.tensor_scalar_add(
                        qt_t[:, f, :], ps_q[:], bqk_sb[:, f : f + 1]
                    )

                v_t = qkvp.tile(
                    [128, tpc, HC, DH + 1], BF, tag="v", bufs=nch, name="v_t"
                )
                vcol = v_t[:, :, :, DH : DH + 1].rearrange(
                    "p a h e -> p (a h e)"
                )
                nc.vector.tensor_copy(
                    vcol, onec_f[:, 0:1].broadcast_to([128, tpc * HC])
                )
                for tt in range(tpc):
                    ps_v = psp.tile([128, FQ], F32, tag="F", bufs=1, name="ps_v")
                    for k in range(NK):
                        nc.tensor.matmul(
                            ps_v[:],
                            (xt_t[:, k, ts(tt, 128)]),
                            (wv_sb[:, k, :]),
                            start=(k == 0), stop=(k == NK - 1),
                        )
                    nc.vector.tensor_add(
                        v_t[:, tt, :, 0:DH],
                        ps_v[:, :].rearrange("p (h e) -> p h e", h=HC),
                        bv_bc[:, :].rearrange("p (h e) -> p h e", h=HC),
                    )
                kt_tiles.append(kt_t)
                qt_tiles.append(qt_t)
                v_tiles.append(v_t)
                for g in (2 * c, 2 * c + 1):
                    if g < ntt // 2:
                        for h in (0, 1):
                            attn_group(0, h, g, po0, kp0)

            for h in (0, 1):
                attn_tail(0, h, po0)

            # projection weights: first use is phase C, load after phase A
            wp_sb = constp.tile([128, FQ // 128, D], BF, name="wp_sb")
            nc.sync.dma_start(wp_sb[:], wp_r[:, :, :])

            def emit_proj(cp):
                for tt in range(tpc):
                    tglob = cp * tpc + tt
                    o_t = workp.tile(
                        [128, D], F32, tag="out", bufs=2, name="o_t"
                    )
                    for n in range(D // CHUNK):
                        ps_f = psp.tile(
                            [128, CHUNK], F32, tag="F", bufs=1, name="ps_f"
                        )
                        for j in range(FQ // 128):
                            nc.tensor.matmul(
                                ps_f[:],
                                (ot_stk[:, j, ts(tglob, 128)]),
                                (wp_sb[:, j, ts(n, CHUNK)]),
                                start=(j == 0), stop=(j == FQ // 128 - 1),
                            )
                        nc.vector.tensor_copy(o_t[:, ts(n, CHUNK)], ps_f[:])
                    nc.sync.dma_start(out_d[ts(tglob, 128), :], o_t[:])

            # ---- phase B: attention per (chunk, head); phase C: projection
            # (each chunk's projection is emitted one chunk late, mid-pair,
            # so the PE never stalls on the normalize/restack chain) ----
            for c in range(nch):
                if c == 0:
                    kp_t = kp0
                else:
                    kp_t = streamp.tile(
                        [128, ntt, CHUNK], BF, tag="kp", bufs=2, name="kp_t"
                    )
                    nc.gpsimd.dma_start(kp_t[:], kp_r[:, :, ts(c, CHUNK)])

                # two independent head chains interleaved per group: while
                # one head's tile sits in the ACT/DVE hop, the PE runs the
                # other head's matmuls (hides cross-engine latency)
                for hp in range(HC // 2):
                    if c == 0 and hp == 0:
                        continue  # interleaved into phase A
                    pair = (2 * hp, 2 * hp + 1)
                    po = {}
                    for h in pair:
                        po[h] = psp.tile(
                            [128, CHUNK], F32, tag="O", bufs=3, name=f"po{h}"
                        )
                    # software-pipeline: V-matmuls run one group behind the
                    # S->exp->mask chain, so the PE never waits on the exp or
                    # mask-mul it just fed (throughput- not latency-limited)
                    ngr = ntt // 2
                    vlag = 1
                    eprev = {}
                    for g in range(ngr + vlag):
                        for h in pair:
                            hb = 64 * (h % 2)
                            hf = h // 2
                            if g < ngr:
                                ps_s = psp.tile(
                                    [128, 2, CHUNK], F32, tag="S", name="ps_s"
                                )
                                for j in range(2):
                                    tkt = 2 * g + j
                                    cc, tt = divmod(tkt, tpc)
                                    nc.tensor.matmul(
                                        ps_s[:, j, :],
                                        (kt_tiles[cc][hb : hb + 64, hf, ts(tt, 128)]),
                                        (qt_tiles[c][hb : hb + 64, hf, :]),
                                        start=True, stop=True,
                                    )
                            if g >= vlag:
                                gp = g - vlag
                                e_p = eprev[(h, gp)]
                                for j in range(2):
                                    tkt = 2 * gp + j
                                    cc, tt = divmod(tkt, tpc)
                                    nc.tensor.matmul(
                                        po[h][0 : DH + 1, :],
                                        (v_tiles[cc][:, tt, h, :]),
                                        (e_p[:, j, :]),
                                        start=(tkt == 0), stop=(tkt == ntt - 1),
                                    )
                            if g < ngr:
                                e_t = workp.tile(
                                    [128, 2, CHUNK], BF, tag="e", bufs=6,
                                    name="e_t",
                                )
                                nc.scalar.activation(
                                    e_t[:], ps_s[:], EXP, scale=0.125
                                )
                                meng = _mask_eng(nc, h, g)
                                meng.tensor_mul(
                                    e_t[:], e_t[:],
                                    kp_t[:, 2 * g : 2 * g + 2, :],
                                )
                                eprev[(h, g)] = e_t
                    for h in pair:
                        hb = 64 * (h % 2)
                        hf = h // 2
                        # normalize: broadcast d across dh partitions, 1/d, mul
                        d_sb = workp.tile(
                            [128, CHUNK], F32R, tag="dsb", bufs=2, name="d_sb"
                        )
                        nc.vector.tensor_copy(
                            d_sb[DH : DH + 1, :], po[h][DH : DH + 1, :]
                        )
                        pb = psp.tile(
                            [64, CHUNK], F32, tag="F", bufs=1, name="pb"
                        )
                        nc.tensor.matmul(
                            pb[:],
                            (ones_col[DH : DH + 1, :]),
                            (d_sb[DH : DH + 1, :]),
                            start=True, stop=True,
                        )
                        pb_sb = workp.tile(
                            [64, CHUNK], F32, tag="pbs", bufs=2, name="pb_sb"
                        )
                        nc.vector.tensor_copy(pb_sb[:], pb[:])
                        rb = workp.tile(
                            [64, CHUNK], F32, tag="rb", bufs=2, name="rb"
                        )
                        nc.vector.reciprocal_approx_fast(rb[:], pb_sb[:])
                        ot_sb = workp.tile(
                            [64, CHUNK], BF, tag="ot", bufs=2, name="ot_sb"
                        )
                        nc.vector.tensor_mul(ot_sb[:], po[h][0:DH, :], rb[:])
                        # restack [64, h] -> [128, h//2] rows for the projection
                        nc.sync.dma_start(
                            ot_stk[hb : hb + 64, hf, ts(c, CHUNK)], ot_sb[:]
                        )

                emit_proj(c)


    nc.compile()
    return nc


def shard_inputs(x, mask, Wqkv, bqkv, Wproj):
    """Build the 8 per-core input maps from full inputs."""
    x = np.asarray(x, dtype=np.float32)
    Wqkv = np.asarray(Wqkv, dtype=np.float32)
    bqkv = np.asarray(bqkv, dtype=np.float32)
    keept = (np.asarray(mask)[0, 0].T == 0).astype(ml_dtypes.bfloat16)
    in_maps = []
    for c in range(8):
        b, g = divmod(c, 4)
        q0 = g * FQ
        wqk = np.concatenate(
            [Wqkv[:, q0 : q0 + FQ], Wqkv[:, D + q0 : D + q0 + FQ]], axis=1
        )
        bqk = np.concatenate(
            [bqkv[q0 : q0 + FQ], bqkv[D + q0 : D + q0 + FQ]]
        ).reshape(4, 128).T
        in_maps.append({
            "xt": np.ascontiguousarray(x[b].T).astype(ml_dtypes.bfloat16),
            "wqk": np.ascontiguousarray(wqk).astype(ml_dtypes.bfloat16),
            "wv": np.ascontiguousarray(
                Wqkv[:, 2 * D + q0 : 2 * D + q0 + FQ]
            ).astype(ml_dtypes.bfloat16),
            "wp": np.ascontiguousarray(
                np.asarray(Wproj, np.float32)[q0 : q0 + FQ, :]
            ).astype(ml_dtypes.bfloat16),
            "bqk": np.ascontiguousarray(bqk),
            "bv": bqkv[2 * D + q0 : 2 * D + q0 + FQ].reshape(1, FQ).copy(),
            "keept": keept,
        })
    return in_maps


_PROGRAM = None
WARM_MMS = int(os.environ.get("KERNEL_WARM_MMS", "0"))


def _get_program():
    global _PROGRAM
    if _PROGRAM is None:
        _PROGRAM = build_program(T_FULL, warm_mms=WARM_MMS)
    return _PROGRAM


def run_on_hw(in_maps, **kwargs):
    from concourse.bass_utils import run_bass_kernel_spmd

    nc = _get_program()
    return run_bass_kernel_spmd(nc, in_maps, list(range(8)), **kwargs)


def gather_output(results, bproj):
    parts = [results[c]["out"] for c in range(8)]
    out = np.stack([
        parts[0] + parts[1] + parts[2] + parts[3],
        parts[4] + parts[5] + parts[6] + parts[7],
    ])
    return (out + np.asarray(bproj, np.float32).reshape(1, 1, D)).astype(np.float32)


def kernel(x, mask, Wqkv, bqkv, Wproj, bproj):
    in_maps = shard_inputs(x, mask, Wqkv, bqkv, Wproj)
    res = run_on_hw(in_maps)
    return gather_output(res.results, bproj)


# revision 14
# speedup vs baseline: 1.2511x; 1.0137x over previous
"""Trainium2 Bass kernel for MultiHeadSelfAttention.

Full inputs -> shard across 8 NeuronCores (batch x head-group) -> SPMD Bass
kernel -> gather partial projections on host.

Per-core shard (core c): batch b = c//4, head group g = c%4 (4 heads of 16).
Device program (per core, T=2048, D=1024, 4 heads, dh=64):
  phase A (per 512-col chunk): qT,kT = (Wqk.T @ x chunks) [feat,T] layout,
                               v     = x @ Wv (natural [T,feat]) + ones col
  phase B (per head, per tq-chunk): S^T[tk,tq] = kT.T-slice @ qT chunk,
    E = exp(S/8) * keepT (bf16 multiplicative mask on DVE 2x-packed mode /
    GpSimd, load-balanced ~5:3), O^T[65,tq] accumulated over tk tiles with a
    ones-row giving the softmax denominator d; normalize rows 0:64 by 1/d
    (K=1 broadcast matmul + fast reciprocal) and DMA-restack heads to
    [128, 2, T] for the projection.
    Two heads' chains are interleaved per group to hide cross-engine latency;
    the K=64 score matmuls of the pair sit at base partitions 0/64 so they
    row-tile (64x128 mode) and run concurrently on the PE.
  phase C (per chunk): out[t,:] = O_cat^T.T @ Wp slices -> DMA out.
All matmuls run in bf16 (same 1 cyc/row as fp32r at free>=256 but half the
operand width/power -> avoids the K=4/8 HAM power throttle that capped the
fp32r version at ~1.2GHz effective, and enables FWL weight-load hiding).
PSUM accumulation stays fp32; softmax denominator path stays fp32.
"""

import os
import sys

import numpy as np

sys.path.insert(0, "/opt/trn_rl_repo")

import ml_dtypes  # noqa: E402

import concourse.mybir as mybir  # noqa: E402
import concourse.tile as tile  # noqa: E402
from concourse import bacc  # noqa: E402
from concourse.bass import ts  # noqa: E402

D = 1024  # d_model
HC = 4  # heads per core
DH = 64  # head dim
FQ = HC * DH  # 256: per-core q (or k or v) feature count
T_FULL = 2048
CHUNK = 512
NK = D // 128  # contraction tiles over d_model

F32 = mybir.dt.float32
F32R = mybir.dt.float32r
BF = mybir.dt.bfloat16
EXP = mybir.ActivationFunctionType.Exp


def _mask_eng(nc, h, g):
    # ~5:3 DVE:GpSimd split of the mask multiplies (GpSimd is ~2.5x slower
    # per element than DVE 2x-packed bf16; DVE also owns the PSUM-side ops)
    return nc.gpsimd if ((g + 3 * h) % 8) >= 5 else nc.vector


def build_program(T=T_FULL, warm_mms=0):
    nch = T // CHUNK  # tq/t chunks
    ntt = T // 128  # t tiles
    tpc = CHUNK // 128  # t tiles per chunk

    nc = bacc.Bacc("TRN2", target_bir_lowering=False, debug=False)

    xt_d = nc.dram_tensor("xt", [D, T], BF, kind="ExternalInput")
    wqk_d = nc.dram_tensor("wqk", [D, 2 * FQ], BF, kind="ExternalInput")
    wv_d = nc.dram_tensor("wv", [D, FQ], BF, kind="ExternalInput")
    wp_d = nc.dram_tensor("wp", [FQ, D], BF, kind="ExternalInput")
    bqk_d = nc.dram_tensor("bqk", [128, 4], F32, kind="ExternalInput")
    bv_d = nc.dram_tensor("bv", [1, FQ], F32R, kind="ExternalInput")
    kp_d = nc.dram_tensor("keept", [T, T], BF, kind="ExternalInput")
    out_d = nc.dram_tensor("out", [T, D], F32, kind="ExternalOutput")

    xt_r = xt_d.rearrange("(a p) t -> p a t", p=128)  # [128, NK, T]
    wqk_r = wqk_d.rearrange("(a p) f -> p a f", p=128)  # [128, NK, 512]
    wv_r = wv_d.rearrange("(a p) f -> p a f", p=128)  # [128, NK, 256]
    wp_r = wp_d.rearrange("(a p) n -> p a n", p=128)  # [128, 2, D]
    kp_r = kp_d.rearrange("(a p) q -> p a q", p=128)  # [128, ntt, T]

    with tile.TileContext(nc) as tc:
        with (
            tc.tile_pool(name="const", bufs=1) as constp,
            tc.tile_pool(name="stream", bufs=2) as streamp,
            tc.tile_pool(name="qkv", bufs=1) as qkvp,
            tc.tile_pool(name="work", bufs=2) as workp,
            tc.tile_pool(name="ps", bufs=2, space="PSUM") as psp,
        ):
            # ---- constants / weights ----
            ones_f = constp.tile([1, 128], F32, name="ones_f")
            nc.vector.memset(ones_f[:], 1.0)
            ones = constp.tile([1, 128], F32R, name="ones")
            nc.vector.tensor_copy(ones[:], ones_f[:])
            onec_f = constp.tile([128, 1], F32, name="onec_f")
            nc.vector.memset(onec_f[:], 1.0)
            # all-ones fp32r rows; row 64 pairs with the PSUM d-row base
            ones_col = constp.tile([128, DH], F32R, name="ones_col")
            nc.vector.tensor_copy(
                ones_col[:], onec_f[:, 0:1].broadcast_to([128, DH])
            )
            bqk_sb = constp.tile([128, 4], F32, name="bqk_sb")
            nc.sync.dma_start(bqk_sb[:], bqk_d[:, :])
            bv_sb = constp.tile([1, FQ], F32R, name="bv_sb")
            nc.sync.dma_start(bv_sb[:], bv_d[:, :])
            wqk_sb = constp.tile([128, NK, 2 * FQ], BF, name="wqk_sb")
            nc.sync.dma_start(
                wqk_sb[:, 0 : NK // 4, :], wqk_r[:, 0 : NK // 4, :]
            )
            nc.sync.dma_start(
                wqk_sb[:, NK // 4 : NK // 2, :],
                wqk_r[:, NK // 4 : NK // 2, :],
            )
            nc.gpsimd.dma_start(
                wqk_sb[:, NK // 2 :, :], wqk_r[:, NK // 2 :, :]
            )
            wv_sb = constp.tile([128, NK, FQ], BF, name="wv_sb")
            nc.gpsimd.dma_start(wv_sb[:], wv_r[:, :, :])
            # broadcast v-bias row across partitions via K=1 matmul
            ps_bb = psp.tile([128, FQ], F32, tag="O", bufs=3, name="ps_bb")
            nc.tensor.matmul(
                ps_bb[:], (ones[0:1, 0:128]), (bv_sb[0:1, :]),
                start=True, stop=True,
            )
            bv_bc = constp.tile([128, FQ], F32, name="bv_bc")
            nc.vector.tensor_copy(bv_bc[:], ps_bb[:])

            ot_stk = constp.tile([128, FQ // 128, T], BF, name="ot_stk")

            def attn_group(c, h, g, po, kp_t):
                hb = 64 * (h % 2)
                hf = h // 2
                ps_s = psp.tile([128, 2, CHUNK], F32, tag="S", name="ps_s")
                for j in range(2):
                    tkt = 2 * g + j
                    cc, tt = divmod(tkt, tpc)
                    nc.tensor.matmul(
                        ps_s[:, j, :],
                        (kt_tiles[cc][hb : hb + 64, hf, ts(tt, 128)]),
                        (qt_tiles[c][hb : hb + 64, hf, :]),
                        start=True, stop=True,
                    )
                e_t = workp.tile(
                    [128, 2, CHUNK], BF, tag="e", bufs=6, name="e_t"
                )
                nc.scalar.activation(e_t[:], ps_s[:], EXP, scale=0.125)
                meng = _mask_eng(nc, h, g)
                meng.tensor_mul(e_t[:], e_t[:], kp_t[:, 2 * g : 2 * g + 2, :])
                for j in range(2):
                    tkt = 2 * g + j
                    cc, tt = divmod(tkt, tpc)
                    nc.tensor.matmul(
                        po[h][:, :],
                        (v_tiles[cc][:, tt, h, :]),
                        (e_t[:, j, :]),
                        start=(tkt == 0), stop=(tkt == ntt - 1),
                    )

            def attn_tail(c, h, po):
                hb = 64 * (h % 2)
                hf = h // 2
                d_sb = workp.tile(
                    [128, CHUNK], F32R, tag="dsb", bufs=2, name="d_sb"
                )
                nc.vector.tensor_copy(
                    d_sb[DH : DH + 1, :], po[h][DH : DH + 1, :]
                )
                pb = psp.tile([64, CHUNK], F32, tag="F", bufs=1, name="pb")
                nc.tensor.matmul(
                    pb[:],
                    (ones_col[DH : DH + 1, :]),
                    (d_sb[DH : DH + 1, :]),
                    start=True, stop=True,
                )
                pb_sb = workp.tile(
                    [64, CHUNK], F32, tag="pbs", bufs=2, name="pb_sb"
                )
                nc.vector.tensor_copy(pb_sb[:], pb[:])
                rb = workp.tile([64, CHUNK], F32, tag="rb", bufs=2, name="rb")
                nc.vector.reciprocal_approx_fast(rb[:], pb_sb[:])
                ot_sb = workp.tile(
                    [64, CHUNK], BF, tag="ot", bufs=2, name="ot_sb"
                )
                nc.vector.tensor_mul(ot_sb[:], po[h][0:DH, :], rb[:])
                nc.sync.dma_start(
                    ot_stk[hb : hb + 64, hf, ts(c, CHUNK)], ot_sb[:]
                )

            # chunk-0 pair-0 runs interleaved with phase A (ACT is idle there)
            # kp0 follows the weight loads on the gpsimd queue, split in 4 so
            # the early mask groups land before the whole 2MB finishes
            kp0 = streamp.tile(
                [128, ntt, CHUNK], BF, tag="kp", bufs=2, name="kp0"
            )
            for q in range(4):
                nc.gpsimd.dma_start(
                    kp0[:, 4 * q : 4 * q + 4, :],
                    kp_r[:, 4 * q : 4 * q + 4, ts(0, CHUNK)],
                )
            po0 = {}
            for h in (0, 1):
                po0[h] = psp.tile(
                    [128, CHUNK], F32, tag="O", bufs=3, name=f"po0_{h}"
                )

            # ---- phase A: qT, kT (transposed) and v (natural) ----
            kt_tiles, qt_tiles, v_tiles = [], [], []
            for c in range(nch):
                xt_t = streamp.tile(
                    [128, NK, CHUNK], BF, tag="xt", bufs=2, name="xt_t"
                )
                nc.scalar.dma_start(
                    xt_t[:, 0 : NK // 2, :],
                    xt_r[:, 0 : NK // 2, ts(c, CHUNK)],
                )
                nc.sync.dma_start(
                    xt_t[:, NK // 2 :, :],
                    xt_r[:, NK // 2 :, ts(c, CHUNK)],
                )

                kt_t = qkvp.tile(
                    [128, 2, CHUNK], BF, tag="kt", bufs=nch, name="kt_t"
                )
                qt_t = qkvp.tile(
                    [128, 2, CHUNK], BF, tag="qt", bufs=nch, name="qt_t"
                )
                for f in range(2):  # k features: wqk cols 256..511
                    ps_k = psp.tile([128, CHUNK], F32, tag="S", name="ps_k")
                    for k in range(NK):
                        nc.tensor.matmul(
                            ps_k[:],
                            (wqk_sb[:, k, ts(2 + f, 128)]),
                            (xt_t[:, k, :]),
                            start=(k == 0), stop=(k == NK - 1),
                        )
                    nc.vector.tensor_scalar_add(
                        kt_t[:, f, :], ps_k[:], bqk_sb[:, 2 + f : 3 + f]
                    )
                for f in range(2):  # q features: wqk cols 0..255
                    ps_q = psp.tile([128, CHUNK], F32, tag="S", name="ps_q")
                    for k in range(NK):
                        nc.tensor.matmul(
                            ps_q[:],
                            (wqk_sb[:, k, ts(f, 128)]),
                            (xt_t[:, k, :]),
                            start=(k == 0), stop=(k == NK - 1),
                        )
                    nc.vector.tensor_scalar_add(
                        qt_t[:, f, :], ps_q[:], bqk_sb[:, f : f + 1]
                    )

                # v padded to 128 feature columns: col DH is the ones column
                # (softmax denominator row), cols DH+1.. are ones-filler so
                # the AV lhsT is a full 128-col weight load (enables FWL --
                # without it every AV matmul pays ~110ns exposed LDWEIGHTS)
                v_t = qkvp.tile(
                    [128, tpc, HC, 128], BF, tag="v", bufs=nch, name="v_t"
                )
                nc.vector.memset(v_t[:, :, :, DH:], 1.0)
                for tt in range(tpc):
                    ps_v = psp.tile([128, FQ], F32, tag="F", bufs=1, name="ps_v")
                    for k in range(NK):
                        nc.tensor.matmul(
                            ps_v[:],
                            (xt_t[:, k, ts(tt, 128)]),
                            (wv_sb[:, k, :]),
                            start=(k == 0), stop=(k == NK - 1),
                        )
                    nc.vector.tensor_add(
                        v_t[:, tt, :, 0:DH],
                        ps_v[:, :].rearrange("p (h e) -> p h e", h=HC),
                        bv_bc[:, :].rearrange("p (h e) -> p h e", h=HC),
                    )
                kt_tiles.append(kt_t)
                qt_tiles.append(qt_t)
                v_tiles.append(v_t)
                for g in (2 * c, 2 * c + 1):
                    if g < ntt // 2:
                        for h in (0, 1):
                            attn_group(0, h, g, po0, kp0)

            for h in (0, 1):
                attn_tail(0, h, po0)

            # projection weights: first use is phase C, load after phase A
            wp_sb = constp.tile([128, FQ // 128, D], BF, name="wp_sb")
            nc.sync.dma_start(wp_sb[:], wp_r[:, :, :])

            def emit_proj_tile(cp, tt):
                tglob = cp * tpc + tt
                o_t = workp.tile(
                    [128, D], F32, tag="out", bufs=2, name="o_t"
                )
                for n in range(D // CHUNK):
                    ps_f = psp.tile(
                        [128, CHUNK], F32, tag="F", bufs=1, name="ps_f"
                    )
                    for j in range(FQ // 128):
                        nc.tensor.matmul(
                            ps_f[:],
                            (ot_stk[:, j, ts(tglob, 128)]),
                            (wp_sb[:, j, ts(n, CHUNK)]),
                            start=(j == 0), stop=(j == FQ // 128 - 1),
                        )
                    nc.vector.tensor_copy(o_t[:, ts(n, CHUNK)], ps_f[:])
                nc.sync.dma_start(out_d[ts(tglob, 128), :], o_t[:])

            # projection tq-tiles are drained one at a time inside the NEXT
            # chunk's group loop: the PE queue is in-order, so emitting them
            # in a block right after the tails would stall the PE on the
            # normalize/restack DMA chain
            pending_proj = []

            # ---- phase B: attention per (chunk, head); phase C: projection
            # (each chunk's projection is emitted one chunk late, mid-pair,
            # so the PE never stalls on the normalize/restack chain) ----
            for c in range(nch):
                if c == 0:
                    kp_t = kp0
                else:
                    kp_t = streamp.tile(
                        [128, ntt, CHUNK], BF, tag="kp", bufs=2, name="kp_t"
                    )
                    nc.gpsimd.dma_start(kp_t[:], kp_r[:, :, ts(c, CHUNK)])

                # two independent head chains interleaved per group: while
                # one head's tile sits in the ACT/DVE hop, the PE runs the
                # other head's matmuls (hides cross-engine latency)
                for hp in range(HC // 2):
                    if c == 0 and hp == 0:
                        continue  # interleaved into phase A
                    pair = (2 * hp, 2 * hp + 1)
                    po = {}
                    for h in pair:
                        po[h] = psp.tile(
                            [128, CHUNK], F32, tag="O", bufs=3, name=f"po{h}"
                        )
                    # software-pipeline: V-matmuls run one group behind the
                    # S->exp->mask chain, so the PE never waits on the exp or
                    # mask-mul it just fed (throughput- not latency-limited)
                    ngr = ntt // 2
                    vlag = 1
                    eprev = {}
                    for g in range(ngr + vlag):
                        for h in pair:
                            hb = 64 * (h % 2)
                            hf = h // 2
                            if g < ngr:
                                ps_s = psp.tile(
                                    [128, 2, CHUNK], F32, tag="S", name="ps_s"
                                )
                                for j in range(2):
                                    tkt = 2 * g + j
                                    cc, tt = divmod(tkt, tpc)
                                    nc.tensor.matmul(
                                        ps_s[:, j, :],
                                        (kt_tiles[cc][hb : hb + 64, hf, ts(tt, 128)]),
                                        (qt_tiles[c][hb : hb + 64, hf, :]),
                                        start=True, stop=True,
                                    )
                            if g >= vlag:
                                gp = g - vlag
                                e_p = eprev[(h, gp)]
                                for j in range(2):
                                    tkt = 2 * gp + j
                                    cc, tt = divmod(tkt, tpc)
                                    nc.tensor.matmul(
                                        po[h][:, :],
                                        (v_tiles[cc][:, tt, h, :]),
                                        (e_p[:, j, :]),
                                        start=(tkt == 0), stop=(tkt == ntt - 1),
                                    )
                            if g < ngr:
                                e_t = workp.tile(
                                    [128, 2, CHUNK], BF, tag="e", bufs=6,
                                    name="e_t",
                                )
                                nc.scalar.activation(
                                    e_t[:], ps_s[:], EXP, scale=0.125
                                )
                                meng = _mask_eng(nc, h, g)
                                meng.tensor_mul(
                                    e_t[:], e_t[:],
                                    kp_t[:, 2 * g : 2 * g + 2, :],
                                )
                                eprev[(h, g)] = e_t
                        if pending_proj and g % 2 == 1:
                            emit_proj_tile(*pending_proj.pop(0))
                    for h in pair:
                        hb = 64 * (h % 2)
                        hf = h // 2
                        # normalize: broadcast d across dh partitions, 1/d, mul
                        d_sb = workp.tile(
                            [128, CHUNK], F32R, tag="dsb", bufs=2, name="d_sb"
                        )
                        nc.vector.tensor_copy(
                            d_sb[DH : DH + 1, :], po[h][DH : DH + 1, :]
                        )
                        pb = psp.tile(
                            [64, CHUNK], F32, tag="F", bufs=1, name="pb"
                        )
                        nc.tensor.matmul(
                            pb[:],
                            (ones_col[DH : DH + 1, :]),
                            (d_sb[DH : DH + 1, :]),
                            start=True, stop=True,
                        )
                        pb_sb = workp.tile(
                            [64, CHUNK], F32, tag="pbs", bufs=2, name="pb_sb"
                        )
                        nc.vector.tensor_copy(pb_sb[:], pb[:])
                        rb = workp.tile(
                            [64, CHUNK], F32, tag="rb", bufs=2, name="rb"
                        )
                        nc.vector.reciprocal_approx_fast(rb[:], pb_sb[:])
                        ot_sb = workp.tile(
                            [64, CHUNK], BF, tag="ot", bufs=2, name="ot_sb"
                        )
                        nc.vector.tensor_mul(ot_sb[:], po[h][0:DH, :], rb[:])
                        # restack [64, h] -> [128, h//2] rows for the projection
                        nc.sync.dma_start(
                            ot_stk[hb : hb + 64, hf, ts(c, CHUNK)], ot_sb[:]
                        )

                pending_proj.extend((c, tt) for tt in range(tpc))

            while pending_proj:
                emit_proj_tile(*pending_proj.pop(0))


    nc.compile()
    return nc


def shard_inputs(x, mask, Wqkv, bqkv, Wproj):
    """Build the 8 per-core input maps from full inputs."""
    x = np.asarray(x, dtype=np.float32)
    Wqkv = np.asarray(Wqkv, dtype=np.float32)
    bqkv = np.asarray(bqkv, dtype=np.float32)
    keept = (np.asarray(mask)[0, 0].T == 0).astype(ml_dtypes.bfloat16)
    in_maps = []
    for c in range(8):
        b, g = divmod(c, 4)
        q0 = g * FQ
        wqk = np.concatenate(
            [Wqkv[:, q0 : q0 + FQ], Wqkv[:, D + q0 : D + q0 + FQ]], axis=1
        )
        bqk = np.concatenate(
            [bqkv[q0 : q0 + FQ], bqkv[D + q0 : D + q0 + FQ]]
        ).reshape(4, 128).T
        in_maps.append({
            "xt": np.ascontiguousarray(x[b].T).astype(ml_dtypes.bfloat16),
            "wqk": np.ascontiguousarray(wqk).astype(ml_dtypes.bfloat16),
            "wv": np.ascontiguousarray(
                Wqkv[:, 2 * D + q0 : 2 * D + q0 + FQ]
            ).astype(ml_dtypes.bfloat16),
            "wp": np.ascontiguousarray(
                np.asarray(Wproj, np.float32)[q0 : q0 + FQ, :]
            ).astype(ml_dtypes.bfloat16),
            "bqk": np.ascontiguousarray(bqk),
            "bv": bqkv[2 * D + q0 : 2 * D + q0 + FQ].reshape(1, FQ).copy(),
            "keept": keept,
        })
    return in_maps


_PROGRAM = None
WARM_MMS = int(os.environ.get("KERNEL_WARM_MMS", "0"))


def _get_program():
    global _PROGRAM
    if _PROGRAM is None:
        _PROGRAM = build_program(T_FULL, warm_mms=WARM_MMS)
    return _PROGRAM


def run_on_hw(in_maps, **kwargs):
    from concourse.bass_utils import run_bass_kernel_spmd

    nc = _get_program()
    return run_bass_kernel_spmd(nc, in_maps, list(range(8)), **kwargs)


def gather_output(results, bproj):
    parts = [results[c]["out"] for c in range(8)]
    out = np.stack([
        parts[0] + parts[1] + parts[2] + parts[3],
        parts[4] + parts[5] + parts[6] + parts[7],
    ])
    return (out + np.asarray(bproj, np.float32).reshape(1, 1, D)).astype(np.float32)


def kernel(x, mask, Wqkv, bqkv, Wproj, bproj):
    in_maps = shard_inputs(x, mask, Wqkv, bqkv, Wproj)
    res = run_on_hw(in_maps)
    return gather_output(res.results, bproj)
